# revision 14
# baseline (speedup 1.0000x reference)
"""Trainium2 Bass kernel for nn_LogicGatedSpikingSelfAttention.

The attention has no softmax, so it is linear:
    x_attn = scale * gate * q @ (k^T v)
O(N*hd^2) instead of O(N^2*hd), with every intermediate an exact small
integer count (spikes are {0,1}; fp16 spikes / f32 PSUM are exact).

Sharding:
  Stage 1 (channel-parallel): core c owns output channels 128c..128c+127
  (heads 2c, 2c+1) of the q/k/v branches; BN stats are per-channel over
  all tokens -> local. Spikes threshold straight from PSUM (the linear
  bias cancels against the threshold shift). k/v branches run bank-outer
  so the stats overlap the GEMM tail. M_b = k^T v is built locally from
  PE-transposed spikes, the transposes pipelined into the branch-stat
  boundaries. Per-head energies go through a small AllReduce (placed via
  a scatter matmul) for the logic gate.
  Resharding: one 1 MB AllToAll moves ungated attention spikes from
  [own 128 ch, 4096 tok] to [all 1024 ch, own 512 tok].
  Stage 2 (token-parallel): the gate ({0,1}) folds into Wp^T rows
  in-place during the AllToAll; projection BN stats finish with an 8 KB
  AllReduce of per-channel (sum, sumsq).
"""
import numpy as np
import ml_dtypes

import concourse.bass as bass
import concourse.bacc as bacc
import concourse.tile as tile
from concourse import mybir
from concourse.bass_utils import run_bass_kernel_spmd

NCORES = 8
B, NSEQ, D, H = 4, 1024, 1024, 16
HD = D // H            # 64 head dim
CH = D // NCORES       # 128 channels per core (2 heads)
TOK = B * NSEQ         # 4096 tokens
KT = D // 128          # 8 contraction tiles
LTOK = TOK // NCORES   # 512 local tokens for stage 2
EPS = 1e-5
S_TH = 1.75            # integer threshold: S >= 2^0.75 <=> S >= 1.75
F32 = mybir.dt.float32
BF16 = mybir.dt.bfloat16
FP16 = mybir.dt.float16
BF = ml_dtypes.bfloat16

_CACHE = {}


def _build(for_sim=False):
    nc = bacc.Bacc("TRN2", target_bir_lowering=False, debug=False,
                   num_devices=NCORES)
    inp = {}
    def din(name, shape, dt=BF16):
        inp[name] = nc.dram_tensor(name, shape, dt, kind="ExternalInput")
        return inp[name]

    for kt in range(KT):                  # x pre-tiled per kt chunk
        din(f"xt{kt}", [128, TOK])
    din("wq", [128, KT * CH]); din("wk", [128, KT * CH])
    din("wv", [128, KT * CH])
    din("wpT", [128, KT * 8 * 128])       # [p, kt, mt, m] = Wp[mt*128+m, kt*128+p]
    for nm in ("tq", "tk", "tv"):
        din(nm, [CH, 1], F32)
    din("tp", [128, 8], F32)              # (2-beta_p)/gamma_p per (p, mt)
    din("wgr", [H, H], F32)               # lhsT: [h, h'] = sum_r Wg[h', h+16r]/1024
    din("bgr", [H, 1], F32)
    din("sel2T", [2, H], F32)             # per-core: [j, h] = (h == 2c+j)
    din("bmask", [H, B], F32)             # per-core: [h, b] = (b == c//2)
    din("mask16", [H, 8], F32)            # [h, kt] = (h//2 == kt)
    din("sel16", [H, 128], F32)           # [h, p] = (h%2 == p//64)
    din("idn", [128, 128], FP16)          # identity for PE transpose
    outT = nc.dram_tensor("outT", [8, 128, LTOK], BF16, kind="ExternalOutput")

    with tile.TileContext(nc) as tc:
        with tc.tile_pool(name="consts", bufs=1) as consts, \
             tc.tile_pool(name="spikes", bufs=1) as spk_pool, \
             tc.tile_pool(name="dram", bufs=1, space="DRAM") as dram:
            _body(tc, inp, outT, consts, spk_pool, dram)
    if for_sim:
        nc.insert_bir_kernel_barrier_sem_inc()
    else:
        nc.compile()
    return nc


def _body(tc, inp, outT, consts, spk_pool, dram):
    nc = tc.nc
    V, SC, GP, TE = nc.vector, nc.scalar, nc.gpsimd, nc.tensor
    AF = mybir.ActivationFunctionType
    OP = mybir.AluOpType
    DENG = [nc.sync, nc.scalar, nc.gpsimd]

    # ---- DRAM scratch for the collectives ----
    e_pay = dram.tile([H * B], F32)
    e_red = dram.tile([H * B], F32, addr_space="Shared")
    a2a_pay = dram.tile([NCORES * 128 * LTOK], BF16)
    a2a_out = dram.tile([NCORES * 128 * LTOK], BF16)
    st_pay = dram.tile([128 * 8 * 2], F32)
    st_out = dram.tile([128 * 8 * 2], F32, addr_space="Shared")

    # ---- constants / weights to SBUF ----
    small = {}
    for nm in ("tq", "tk", "tv", "bgr"):
        t = consts.tile([inp[nm].shape[0], 1], F32, name=f"{nm}_sb")
        nc.sync.dma_start(t[:], inp[nm].ap())
        small[nm] = t
    tp_sb = consts.tile([128, 8], F32)
    nc.sync.dma_start(tp_sb[:], inp["tp"].ap())
    wgr_sb = consts.tile([H, H], F32)
    nc.sync.dma_start(wgr_sb[:], inp["wgr"].ap())
    sel2T_sb = consts.tile([2, H], F32)
    nc.sync.dma_start(sel2T_sb[:], inp["sel2T"].ap())
    bmask_sb = consts.tile([H, B], F32)
    nc.sync.dma_start(bmask_sb[:], inp["bmask"].ap())
    mask16_sb = consts.tile([H, 8], F32)
    nc.sync.dma_start(mask16_sb[:], inp["mask16"].ap())
    sel16_sb = consts.tile([H, 128], F32)
    nc.sync.dma_start(sel16_sb[:], inp["sel16"].ap())
    idn_sb = consts.tile([128, 128], FP16)
    nc.scalar.dma_start(idn_sb[:], inp["idn"].ap())
    eps_sb = consts.tile([128, 1], F32)
    V.memset(eps_sb[:], EPS)
    nsth_sb = consts.tile([128, 1], F32)
    V.memset(nsth_sb[:], -S_TH)
    w_sb = {}
    for i, nm in enumerate(("wq", "wk", "wv")):
        t = consts.tile([128, KT, CH], BF16, name=f"{nm}_sb")
        DENG[i % 3].dma_start(
            t[:], inp[nm].ap().rearrange("p (t m) -> p t m", t=KT))
        w_sb[nm] = t
    xts = []
    for kt in range(KT):
        t = consts.tile([128, TOK], BF16, name=f"xt{kt}_sb")
        DENG[kt % 3].dma_start(t[:], inp[f"xt{kt}"].ap())
        xts.append(t)
    wpT_sb = consts.tile([128, KT, 8, 128], BF16)
    for kt in range(KT):
        DENG[kt % 3].dma_start(
            wpT_sb[:, kt, :, :],
            inp["wpT"].ap().rearrange("p (t m n) -> p t m n",
                                      t=KT, m=8)[:, kt, :, :])

    # ---- persistent spike tensors (fp16: {0,1} exact) ----
    sp = {nm: spk_pool.tile([128, TOK], FP16, name=f"sp{nm}")
          for nm in ("q", "k", "v")}
    knat = spk_pool.tile([128, 32, 128], FP16)   # [tok, b*8+t, ch]
    vnat = spk_pool.tile([128, 32, 128], FP16)
    m_sb = spk_pool.tile([128, B, 128], FP16)    # block-diag k^T v per batch
    V.memset(m_sb[:], 0.0)
    xsp_cm = spk_pool.tile([128, TOK], BF16)     # attn spikes (channel-major)
    xsp_tok = spk_pool.tile([128, KT, LTOK], BF16)  # after AllToAll

    def spike_store(dst_aps, ps_list, thr, nthr, stp):
        """dst[i] = (ps[i] >= thr) as {0,1}; chunks 0-3 on V, 4-7 relayed
        SC (psum - thr -> SBUF) -> GP (>= 0)."""
        for i, (dst, psrc) in enumerate(zip(dst_aps, ps_list)):
            if i < len(dst_aps) // 2:
                V.tensor_scalar(dst, psrc, thr[:], None, OP.is_ge)
            else:
                sh = stp.tile([128, 512], F32, tag=f"sh{i % 2}")
                SC.activation(sh[:], psrc, AF.Identity, bias=nthr[:])
                GP.tensor_scalar(dst, sh[:], 0.0, None, OP.is_ge)

    def branch(nm, bank_outer, stp, brps):
        ps = [brps.tile([128, 512], F32, name=f"ps{nm}{i}") for i in range(8)]
        if bank_outer:
            for nck in range(8):
                for kt in range(KT):
                    TE.matmul(ps[nck][:], w_sb["w" + nm][:, kt, :],
                              xts[kt][:, nck * 512:(nck + 1) * 512],
                              start=(kt == 0), stop=(kt == KT - 1))
        else:
            for kt in range(KT):
                for nck in range(8):
                    TE.matmul(ps[nck][:], w_sb["w" + nm][:, kt, :],
                              xts[kt][:, nck * 512:(nck + 1) * 512],
                              start=(kt == 0), stop=(kt == KT - 1))
        stats = stp.tile([128, 8, 6], F32, tag="stats")
        for i in range(8):
            V.bn_stats(stats[:, i, :], ps[i][:])
        mv = stp.tile([128, 2], F32, tag="mv")
        V.bn_aggr(mv[:], stats[:])
        std = stp.tile([128, 1], F32, tag="std")
        SC.activation(std[:], mv[:, 1:2], AF.Sqrt, bias=eps_sb[:])
        thr = stp.tile([128, 1], F32, tag="thr")
        GP.tensor_tensor(thr[:], std[:], small["t" + nm][:], OP.mult)
        GP.tensor_tensor(thr[:], thr[:], mv[:, 0:1], OP.add)
        nthr = stp.tile([128, 1], F32, tag="nthr")
        GP.tensor_scalar(nthr[:], thr[:], -1.0, None, OP.mult)
        spike_store([sp[nm][:, i * 512:(i + 1) * 512] for i in range(8)],
                    [p[:] for p in ps], thr, nthr, stp)

    def transposes(nm, dst, tps):
        src = sp[nm]
        for i in range(32):
            t = tps.tile([128, 128], FP16, tag="t")
            TE.transpose(t[:], src[:, i * 128:(i + 1) * 128], idn_sb[:])
            if i % 2 == 0:
                V.tensor_copy(dst[:, i, :], t[:])
            else:
                SC.activation(dst[:, i, :], t[:], AF.Copy)

    # ================= stage 1: branches + transposes =================
    with tc.tile_pool(name="stps", bufs=2) as stp:
        with tc.tile_pool(name="brq", bufs=1, space="PSUM") as brps:
            branch("q", False, stp, brps)          # DMA-paced: kt-outer
        with tc.tile_pool(name="brk", bufs=1, space="PSUM") as brps:
            branch("k", True, stp, brps)
        with tc.tile_pool(name="tpsk", bufs=4, space="PSUM") as tps:
            transposes("k", knat, tps)

        # energies: counts of q&k spikes per (head, batch); scatter into
        # a zeroed [16, B] tile via sel2T matmul, then AllReduce-add.
        # Fires here so the collective overlaps the v branch.
        i2e = consts.tile([CH, 2], F32)
        V.memset(i2e[0:HD, 0:1], 1.0)
        V.memset(i2e[0:HD, 1:2], 0.0)
        V.memset(i2e[HD:CH, 0:1], 0.0)
        V.memset(i2e[HD:CH, 1:2], 1.0)
        with tc.tile_pool(name="enps", bufs=1, space="PSUM") as enps, \
             tc.tile_pool(name="entmp", bufs=1) as entmp:
            prod = entmp.tile([128, TOK], FP16)
            GP.tensor_tensor(prod[:, 0:2048], sp["q"][:, 0:2048],
                             sp["k"][:, 0:2048], OP.mult)
            V.tensor_tensor(prod[:, 2048:4096], sp["q"][:, 2048:4096],
                            sp["k"][:, 2048:4096], OP.mult)
            ech = entmp.tile([128, B], F32)
            V.reduce_sum(ech[:],
                         prod[:].rearrange("p (b n) -> p b n", b=B),
                         axis=mybir.AxisListType.X)
            e2_ps = enps.tile([2, B], F32)
            TE.matmul(e2_ps[:], i2e[:], ech[:], start=True, stop=True)
            e2 = entmp.tile([2, B], F32)
            SC.activation(e2[:], e2_ps[:], AF.Copy)
            epad_ps = enps.tile([H, B], F32)
            TE.matmul(epad_ps[:], sel2T_sb[:], e2[:], start=True, stop=True)
            epad = entmp.tile([H, B], F32)
            V.tensor_copy(epad[:], epad_ps[:])
            nc.sync.dma_start(
                e_pay[:].rearrange("(p w) -> p w", p=H), epad[:])
        GP.collective_compute("AllReduce", OP.add,
                              ins=[e_pay.opt()], outs=[e_red.opt()],
                              replica_groups=[list(range(NCORES))])

        with tc.tile_pool(name="brv", bufs=1, space="PSUM") as brps:
            branch("v", True, stp, brps)

        # gate from the reduced energies -> gvec[p, kt] in {0,1}
        # (eg DMA issued before any AllToAll DMAs: avoids queue HOL block)
        gvec = spk_pool.tile([128, 8], F32)
        with tc.tile_pool(name="gtps", bufs=1, space="PSUM") as gtps, \
             tc.tile_pool(name="gsb", bufs=1) as gsb:
            eg = gsb.tile([H, B], F32)
            nc.sync.dma_start(eg[:], e_red[:].rearrange("(h b) -> h b", h=H))
            z_ps = gtps.tile([H, B], F32)
            TE.matmul(z_ps[:], wgr_sb[:], eg[:], start=True, stop=True)
            gate = gsb.tile([H, B], F32)
            V.tensor_scalar(gate[:], z_ps[:], small["bgr"][:], 0.5,
                            OP.add, OP.is_ge)
            gown = gsb.tile([H, 1], F32)  # gate for this core's batch
            V.tensor_tensor(gate[:], gate[:], bmask_sb[:], OP.mult)
            V.reduce_sum(gown[:], gate[:], axis=mybir.AxisListType.X)
            g16 = gsb.tile([H, 8], F32)   # [h, kt] = gown[h] * (h//2 == kt)
            V.tensor_scalar(g16[:], mask16_sb[:], gown[:], None, OP.mult)
            gv_ps = gtps.tile([128, 8], F32)
            TE.matmul(gv_ps[:], sel16_sb[:], g16[:], start=True, stop=True)
            V.tensor_copy(gvec[:], gv_ps[:])

        with tc.tile_pool(name="tpsv", bufs=4, space="PSUM") as tps:
            transposes("v", vnat, tps)

    # ================= M = k^T v (local, per batch) =================
    with tc.tile_pool(name="mps", bufs=1, space="PSUM") as mps:
        for b in range(B):
            m_ps = mps.tile([128, 128], F32, name=f"mps{b}")
            for i in range(8):
                TE.matmul(m_ps[:], knat[:, b * 8 + i, :], vnat[:, b * 8 + i, :],
                          start=(i == 0), stop=(i == 7))
            # keep only the per-head diagonal 64-blocks (fp16 exact: <=1024)
            V.tensor_copy(m_sb[0:64, b, 0:64], m_ps[0:64, 0:64])
            SC.activation(m_sb[64:128, b, 64:128], m_ps[64:128, 64:128],
                          AF.Copy)

    # ============ apply: S = M_b^T q, spike at S >= 1.75 ============
    with tc.tile_pool(name="aps", bufs=1, space="PSUM") as aps, \
         tc.tile_pool(name="astp", bufs=2) as astp:
        s_ps = []
        for b in range(B):
            for nh in range(2):
                n0 = b * NSEQ + nh * 512
                p = aps.tile([128, 512], F32, name=f"s{b}{nh}")
                TE.matmul(p[:], m_sb[:, b, :], sp["q"][:, n0:n0 + 512],
                          start=True, stop=True)
                s_ps.append(p)
        sth = astp.tile([128, 1], F32, tag="sth")
        V.memset(sth[:], S_TH)
        spike_store([xsp_cm[:, i * 512:(i + 1) * 512] for i in range(8)],
                    [p[:] for p in s_ps], sth, nsth_sb, astp)

    # ================= AllToAll: reshard to token-parallel =================
    for j in range(NCORES):
        DENG[j % 3].dma_start(
            a2a_pay[j * 128 * LTOK:(j + 1) * 128 * LTOK].rearrange(
                "(p n) -> p n", p=128),
            xsp_cm[:, j * LTOK:(j + 1) * LTOK])
    GP.collective_compute("AllToAll", OP.bypass,
                          ins=[a2a_pay.opt()], outs=[a2a_out.opt()],
                          replica_groups=[list(range(NCORES))])
    for j in range(NCORES):
        DENG[j % 3].dma_start(
            xsp_tok[:, j, :],
            a2a_out[j * 128 * LTOK:(j + 1) * 128 * LTOK].rearrange(
                "(p n) -> p n", p=128))

    # ====== fold the gate into Wp^T rows in place (overlaps AllToAll) ======
    for kt in range(KT):
        eng = V if kt % 2 == 0 else GP
        eng.tensor_scalar(
            wpT_sb[:, kt, :, :].rearrange("p m n -> p (m n)"),
            wpT_sb[:, kt, :, :].rearrange("p m n -> p (m n)"),
            gvec[:, kt:kt + 1], None, OP.mult)

    # ================= projection (token-parallel) =================
    with tc.tile_pool(name="pstat", bufs=1) as pstat:
        with tc.tile_pool(name="ppps", bufs=1, space="PSUM") as ppps:
            pp = [ppps.tile([128, LTOK], F32, name=f"pp{i}") for i in range(8)]
            for kt in range(KT):
                for mt in range(8):
                    TE.matmul(pp[mt][:], wpT_sb[:, kt, mt, :], xsp_tok[:, kt, :],
                              start=(kt == 0), stop=(kt == KT - 1))
            stats = pstat.tile([128, 8, 6], F32)
            mv8 = pstat.tile([128, 8, 2], F32)
            for mt in range(8):
                V.bn_stats(stats[:, mt, :], pp[mt][:])
                V.bn_aggr(mv8[:, mt, :], stats[:, mt, :])
            s1 = pstat.tile([128, 8], F32)
            s2 = pstat.tile([128, 8], F32)
            V.tensor_scalar(s1[:], mv8[:, :, 0], float(LTOK), None, OP.mult)
            V.tensor_tensor(s2[:], mv8[:, :, 0], mv8[:, :, 0], OP.mult)
            V.tensor_tensor(s2[:], s2[:], mv8[:, :, 1], OP.add)
            V.tensor_scalar(s2[:], s2[:], float(LTOK), None, OP.mult)
            nc.sync.dma_start(
                st_pay[0:1024].rearrange("(p m) -> p m", p=128), s1[:])
            nc.sync.dma_start(
                st_pay[1024:2048].rearrange("(p m) -> p m", p=128), s2[:])
            GP.collective_compute("AllReduce", OP.add,
                                  ins=[st_pay.opt()], outs=[st_out.opt()],
                                  replica_groups=[list(range(NCORES))])
            g1 = pstat.tile([128, 8], F32)
            g2 = pstat.tile([128, 8], F32)
            nc.sync.dma_start(
                g1[:], st_out[0:1024].rearrange("(p m) -> p m", p=128))
            nc.scalar.dma_start(
                g2[:], st_out[1024:2048].rearrange("(p m) -> p m", p=128))
            meang = pstat.tile([128, 8], F32)
            V.tensor_scalar(meang[:], g1[:], 1.0 / TOK, None, OP.mult)
            varg = pstat.tile([128, 8], F32)
            V.tensor_scalar(varg[:], g2[:], 1.0 / TOK, None, OP.mult)
            msq2 = pstat.tile([128, 8], F32)
            V.tensor_tensor(msq2[:], meang[:], meang[:], OP.mult)
            V.tensor_tensor(varg[:], varg[:], msq2[:], OP.subtract)
            stdp = pstat.tile([128, 8], F32)
            SC.activation(stdp[:], varg[:], AF.Sqrt, bias=eps_sb[:])
            thrp = pstat.tile([128, 8], F32)
            V.tensor_tensor(thrp[:], stdp[:], tp_sb[:], OP.mult)
            V.tensor_tensor(thrp[:], thrp[:], meang[:], OP.add)
            nthrp = pstat.tile([128, 8], F32)
            GP.tensor_scalar(nthrp[:], thrp[:], -1.0, None, OP.mult)
            osp = pstat.tile([128, 8, LTOK], BF16)
            for mt in range(8):
                if mt < 4:
                    V.tensor_scalar(osp[:, mt, :], pp[mt][:],
                                    thrp[:, mt:mt + 1], None, OP.is_ge)
                else:
                    sh = pstat.tile([128, 512], F32, name=f"psh{mt}")
                    SC.activation(sh[:], pp[mt][:], AF.Identity,
                                  bias=nthrp[:, mt:mt + 1])
                    GP.tensor_scalar(osp[:, mt, :], sh[:], 0.0, None,
                                     OP.is_ge)
                DENG[mt % 3].dma_start(outT.ap()[mt, :, :], osp[:, mt, :])


def _tile_rows(a):
    # (8*128, N) -> (128, 8*N) so the SBUF [p, (t n)] load is contiguous
    n = a.shape[1]
    return np.ascontiguousarray(
        a.reshape(KT, 128, n).transpose(1, 0, 2).reshape(128, KT * n))


def _prep_inputs(inputs):
    x = np.asarray(inputs["x"], np.float32)
    xT = _tile_rows(x.reshape(TOK, D).T.astype(BF))
    Wg = np.asarray(inputs["Wg"], np.float64)
    wgr = (Wg.reshape(H, HD, H).sum(axis=1).T / 1024.0).astype(np.float32)
    wgr = np.ascontiguousarray(wgr)                     # [h, h']
    bgr = np.asarray(inputs["bg"], np.float32).reshape(H, 1)
    sel16 = np.zeros((H, 128), np.float32)
    sel16[0::2, 0:64] = 1.0
    sel16[1::2, 64:128] = 1.0
    mask16 = np.zeros((H, 8), np.float32)
    for h in range(H):
        mask16[h, h // 2] = 1.0
    idn = np.eye(128, dtype=np.float16)
    Wp = np.asarray(inputs["Wp"], np.float32).astype(BF)
    wpT = np.ascontiguousarray(
        Wp.reshape(8, 128, 8, 128).transpose(3, 2, 0, 1)).reshape(128, -1)
    gpf = np.asarray(inputs["gp"], np.float32)
    bepf = np.asarray(inputs["betap"], np.float32)
    tpv = (2.0 - bepf) / gpf                            # [1024] per c_out
    tp = np.ascontiguousarray(tpv.reshape(8, 128).T).astype(np.float32)
    in_maps = []
    for c in range(NCORES):
        sl = slice(CH * c, CH * c + CH)
        sel2T = np.zeros((2, H), np.float32)
        sel2T[0, 2 * c] = 1.0
        sel2T[1, 2 * c + 1] = 1.0
        bmask = np.zeros((H, B), np.float32)
        bmask[:, c // 2] = 1.0
        m = {"wgr": wgr, "bgr": bgr, "sel2T": sel2T, "bmask": bmask,
             "mask16": mask16, "sel16": sel16, "idn": idn, "wpT": wpT,
             "tp": tp}
        for kt in range(KT):
            m[f"xt{kt}"] = np.ascontiguousarray(xT[:, kt * TOK:(kt + 1) * TOK])
        for nm in ("q", "k", "v"):
            W = np.asarray(inputs[f"W{nm}"], np.float32)
            m["w" + nm] = _tile_rows(W[sl, :].T.astype(BF))
            g = np.asarray(inputs[f"g{nm}"], np.float32)[sl]
            be = np.asarray(inputs[f"beta{nm}"], np.float32)[sl]
            m["t" + nm] = ((2.0 - be) / g).reshape(CH, 1).astype(np.float32)
        in_maps.append(m)
    return in_maps


def _assemble(results):
    out = np.empty((TOK, D), np.float32)
    for c in range(NCORES):
        o = np.asarray(results[c]["outT"], dtype=np.float32)  # [8, 128, 512]
        out[LTOK * c:LTOK * (c + 1), :] = \
            o.transpose(2, 0, 1).reshape(LTOK, D)
    return out.reshape(B, NSEQ, D)


def _run(inputs, trace=False):
    if "nc" not in _CACHE:
        _CACHE["nc"] = _build()
    nc = _CACHE["nc"]
    in_maps = _prep_inputs(inputs)
    res = run_bass_kernel_spmd(nc, in_maps, core_ids=list(range(NCORES)),
                               trace=trace)
    return _assemble(res.results), res


def kernel(**inputs) -> np.ndarray:
    out, _ = _run(inputs, trace=False)
    return out


# revision 15
# speedup vs baseline: 1.9326x; 1.9326x over previous
"""Trainium2 Bass kernel for nn_LogicGatedSpikingSelfAttention.

The attention has no softmax, so it is linear:
    x_attn = scale * gate * q @ (k^T v)
O(N*hd^2) instead of O(N^2*hd), with every intermediate an exact small
integer count (spikes are {0,1}; fp16 spikes / f32 PSUM are exact).

Sharding:
  Stage 1 (channel-parallel): core c owns output channels 128c..128c+127
  (heads 2c, 2c+1) of the q/k/v branches; BN stats are per-channel over
  all tokens -> local. Spikes threshold straight from PSUM (the linear
  bias cancels against the threshold shift). k/v branches run bank-outer
  so the stats overlap the GEMM tail. M_b = k^T v is built locally from
  PE-transposed spikes, the transposes pipelined into the branch-stat
  boundaries. Per-head energies go through a small AllReduce (placed via
  a scatter matmul) for the logic gate.
  Resharding: one 1 MB AllToAll moves ungated attention spikes from
  [own 128 ch, 4096 tok] to [all 1024 ch, own 512 tok].
  Stage 2 (token-parallel): the gate ({0,1}) folds into Wp^T rows
  in-place during the AllToAll; projection BN stats finish with an 8 KB
  AllReduce of per-channel (sum, sumsq).
"""
import numpy as np
import ml_dtypes

import concourse.bass as bass
import concourse.bacc as bacc
import concourse.tile as tile
from concourse import mybir
from concourse.bass_utils import run_bass_kernel_spmd

NCORES = 8
B, NSEQ, D, H = 4, 1024, 1024, 16
HD = D // H            # 64 head dim
CH = D // NCORES       # 128 channels per core (2 heads)
TOK = B * NSEQ         # 4096 tokens
KT = D // 128          # 8 contraction tiles
LTOK = TOK // NCORES   # 512 local tokens for stage 2
EPS = 1e-5
S_TH = 1.75            # integer threshold: S >= 2^0.75 <=> S >= 1.75
GATE_OFF = float(2.0 ** 20)    # attn-LIF threshold when gate == 0
F32 = mybir.dt.float32
BF16 = mybir.dt.bfloat16
FP16 = mybir.dt.float16
BF = ml_dtypes.bfloat16

_CACHE = {}


def _build(for_sim=False):
    nc = bacc.Bacc("TRN2", target_bir_lowering=False, debug=False,
                   num_devices=NCORES)
    inp = {}
    def din(name, shape, dt=BF16):
        inp[name] = nc.dram_tensor(name, shape, dt, kind="ExternalInput")
        return inp[name]

    for kt in range(KT):                  # x pre-tiled per kt chunk
        din(f"xt{kt}", [128, TOK])
    din("wq", [128, KT * CH]); din("wk", [128, KT * CH])
    din("wv", [128, KT * CH])
    din("wpT", [128, KT * 8 * 128])       # [p, kt, mt, m] = Wp[mt*128+m, kt*128+p]
    for nm in ("tq", "tk", "tv"):
        din(nm, [CH, 1], F32)
    din("tp", [128, 8], F32)              # (2-beta_p)/gamma_p per (p, mt)
    din("wgr", [H, H], F32)               # lhsT: [h, h'] = sum_r Wg[h', h+16r]/1024
    din("bgr", [H, 1], F32)
    din("sel2T", [2, H], F32)             # per-core: [j, h] = (h == 2c+j)
    din("sel2", [H, 2], F32)              # per-core: [h, j] = (h == 2c+j)
    din("sel128", [2, 128], F32)          # [j, p] = (p//64 == j)
    din("idn", [128, 128], FP16)          # identity for PE transpose
    outT = nc.dram_tensor("outT", [8, 128, LTOK], BF16, kind="ExternalOutput")

    with tile.TileContext(nc) as tc:
        with tc.tile_pool(name="consts", bufs=1) as consts, \
             tc.tile_pool(name="spikes", bufs=1) as spk_pool, \
             tc.tile_pool(name="dram", bufs=1, space="DRAM") as dram:
            _body(tc, inp, outT, consts, spk_pool, dram)
    if for_sim:
        nc.insert_bir_kernel_barrier_sem_inc()
    else:
        nc.compile()
    return nc


def _body(tc, inp, outT, consts, spk_pool, dram):
    nc = tc.nc
    V, SC, GP, TE = nc.vector, nc.scalar, nc.gpsimd, nc.tensor
    AF = mybir.ActivationFunctionType
    OP = mybir.AluOpType
    DENG = [nc.sync, nc.scalar, nc.gpsimd]

    # ---- DRAM scratch for the collectives ----
    e_pay = dram.tile([H * B], F32)
    e_red = dram.tile([H * B], F32, addr_space="Shared")
    a2a_pay = dram.tile([NCORES * 128 * LTOK], BF16)
    a2a_out = dram.tile([NCORES * 128 * LTOK], BF16)
    st_pay = dram.tile([128 * 8 * 2], F32)
    st_out = dram.tile([128 * 8 * 2], F32, addr_space="Shared")

    # ---- constants / weights to SBUF ----
    small = {}
    for nm in ("tq", "tk", "tv", "bgr"):
        t = consts.tile([inp[nm].shape[0], 1], F32, name=f"{nm}_sb")
        nc.sync.dma_start(t[:], inp[nm].ap())
        small[nm] = t
    tp_sb = consts.tile([128, 8], F32)
    nc.sync.dma_start(tp_sb[:], inp["tp"].ap())
    wgr_sb = consts.tile([H, H], F32)
    nc.sync.dma_start(wgr_sb[:], inp["wgr"].ap())
    sel2T_sb = consts.tile([2, H], F32)
    nc.sync.dma_start(sel2T_sb[:], inp["sel2T"].ap())
    sel2_sb = consts.tile([H, 2], F32)
    nc.sync.dma_start(sel2_sb[:], inp["sel2"].ap())
    sel128_sb = consts.tile([2, 128], F32)
    nc.sync.dma_start(sel128_sb[:], inp["sel128"].ap())
    idn_sb = consts.tile([128, 128], FP16)
    nc.scalar.dma_start(idn_sb[:], inp["idn"].ap())
    eps_sb = consts.tile([128, 1], F32)
    V.memset(eps_sb[:], EPS)
    w_sb = {}
    for i, nm in enumerate(("wq", "wk", "wv")):
        t = consts.tile([128, KT, CH], BF16, name=f"{nm}_sb")
        DENG[i % 3].dma_start(
            t[:], inp[nm].ap().rearrange("p (t m) -> p t m", t=KT))
        w_sb[nm] = t
    xts = []
    for kt in range(KT):
        t = consts.tile([128, TOK], BF16, name=f"xt{kt}_sb")
        DENG[kt % 3].dma_start(t[:], inp[f"xt{kt}"].ap())
        xts.append(t)
    wpT_sb = consts.tile([128, KT, 8, 128], BF16)
    for kt in range(KT):
        DENG[kt % 3].dma_start(
            wpT_sb[:, kt, :, :],
            inp["wpT"].ap().rearrange("p (t m n) -> p t m n",
                                      t=KT, m=8)[:, kt, :, :])

    # ---- persistent spike tensors (fp16: {0,1} exact) ----
    sp = {nm: spk_pool.tile([128, TOK], FP16, name=f"sp{nm}")
          for nm in ("q", "k", "v")}
    knat = spk_pool.tile([128, 32, 128], FP16)   # [tok, b*8+t, ch]
    vnat = spk_pool.tile([128, 32, 128], FP16)
    m_sb = spk_pool.tile([128, B, 128], FP16)    # block-diag k^T v per batch
    V.memset(m_sb[:], 0.0)
    xsp_cm = spk_pool.tile([128, TOK], BF16)     # attn spikes (channel-major)
    xsp_tok = spk_pool.tile([128, KT, LTOK], BF16)  # after AllToAll

    def spike_store(dst_aps, ps_list, thr):
        for dst, psrc in zip(dst_aps, ps_list):
            V.tensor_scalar(dst, psrc, thr, None, OP.is_ge)

    def branch(nm, bank_outer, stp, brps):
        ps = [brps.tile([128, 512], F32, name=f"ps{nm}{i}") for i in range(8)]
        if bank_outer:
            for nck in range(8):
                for kt in range(KT):
                    TE.matmul(ps[nck][:], w_sb["w" + nm][:, kt, :],
                              xts[kt][:, nck * 512:(nck + 1) * 512],
                              start=(kt == 0), stop=(kt == KT - 1))
        else:
            for kt in range(KT):
                for nck in range(8):
                    TE.matmul(ps[nck][:], w_sb["w" + nm][:, kt, :],
                              xts[kt][:, nck * 512:(nck + 1) * 512],
                              start=(kt == 0), stop=(kt == KT - 1))
        stats = stp.tile([128, 8, 6], F32, tag="stats")
        for i in range(8):
            V.bn_stats(stats[:, i, :], ps[i][:])
        mv = stp.tile([128, 2], F32, tag="mv")
        V.bn_aggr(mv[:], stats[:])
        std = stp.tile([128, 1], F32, tag="std")
        SC.activation(std[:], mv[:, 1:2], AF.Sqrt, bias=eps_sb[:])
        thr = stp.tile([128, 1], F32, tag="thr")
        V.tensor_tensor(thr[:], std[:], small["t" + nm][:], OP.mult)
        V.tensor_tensor(thr[:], thr[:], mv[:, 0:1], OP.add)
        spike_store([sp[nm][:, i * 512:(i + 1) * 512] for i in range(8)],
                    [p[:] for p in ps], thr[:])

    def transposes(nm, dst, tps):
        src = sp[nm]
        for i in range(32):
            t = tps.tile([128, 128], FP16, tag="t")
            TE.transpose(t[:], src[:, i * 128:(i + 1) * 128], idn_sb[:])
            if i % 2 == 0:
                V.tensor_copy(dst[:, i, :], t[:])
            else:
                SC.activation(dst[:, i, :], t[:], AF.Copy)

    # ================= stage 1: branches + transposes =================
    with tc.tile_pool(name="stps", bufs=2) as stp:
        with tc.tile_pool(name="brq", bufs=1, space="PSUM") as brps:
            branch("q", False, stp, brps)          # DMA-paced: kt-outer
        with tc.tile_pool(name="brk", bufs=1, space="PSUM") as brps:
            branch("k", True, stp, brps)
        with tc.tile_pool(name="tpsk", bufs=4, space="PSUM") as tps:
            transposes("k", knat, tps)

        # energies: counts of q&k spikes per (head, batch); scatter into
        # a zeroed [16, B] tile via sel2T matmul, then AllReduce-add.
        # Fires here so the collective overlaps the v branch.
        i2e = consts.tile([CH, 2], F32)
        V.memset(i2e[0:HD, 0:1], 1.0)
        V.memset(i2e[0:HD, 1:2], 0.0)
        V.memset(i2e[HD:CH, 0:1], 0.0)
        V.memset(i2e[HD:CH, 1:2], 1.0)
        with tc.tile_pool(name="enps", bufs=1, space="PSUM") as enps, \
             tc.tile_pool(name="entmp", bufs=1) as entmp:
            prod = entmp.tile([128, TOK], FP16)
            GP.tensor_tensor(prod[:], sp["q"][:], sp["k"][:], OP.mult)
            ech = entmp.tile([128, B], F32)
            V.reduce_sum(ech[:],
                         prod[:].rearrange("p (b n) -> p b n", b=B),
                         axis=mybir.AxisListType.X)
            e2_ps = enps.tile([2, B], F32)
            TE.matmul(e2_ps[:], i2e[:], ech[:], start=True, stop=True)
            e2 = entmp.tile([2, B], F32)
            SC.activation(e2[:], e2_ps[:], AF.Copy)
            epad_ps = enps.tile([H, B], F32)
            TE.matmul(epad_ps[:], sel2T_sb[:], e2[:], start=True, stop=True)
            epad = entmp.tile([H, B], F32)
            V.tensor_copy(epad[:], epad_ps[:])
            nc.sync.dma_start(
                e_pay[:].rearrange("(p w) -> p w", p=H), epad[:])
        GP.collective_compute("AllReduce", OP.add,
                              ins=[e_pay.opt()], outs=[e_red.opt()],
                              replica_groups=[list(range(NCORES))])

        with tc.tile_pool(name="brv", bufs=1, space="PSUM") as brps:
            branch("v", True, stp, brps)

        with tc.tile_pool(name="tpsv", bufs=4, space="PSUM") as tps:
            transposes("v", vnat, tps)

        # gate -> per-(partition, batch) attn-LIF thresholds thrv[128, B]
        # ({0,1} gate becomes threshold 1.75 (on) / 2^20 (off))
        thrv = spk_pool.tile([128, B], F32)
        with tc.tile_pool(name="gtps", bufs=1, space="PSUM") as gtps, \
             tc.tile_pool(name="gsb", bufs=1) as gsb:
            eg = gsb.tile([H, B], F32)
            nc.sync.dma_start(eg[:], e_red[:].rearrange("(h b) -> h b", h=H))
            z_ps = gtps.tile([H, B], F32)
            TE.matmul(z_ps[:], wgr_sb[:], eg[:], start=True, stop=True)
            gate = gsb.tile([H, B], F32)
            V.tensor_scalar(gate[:], z_ps[:], small["bgr"][:], 0.5,
                            OP.add, OP.is_ge)
            gthr = gsb.tile([H, B], F32)
            V.tensor_scalar(gthr[:], gate[:], S_TH - GATE_OFF, GATE_OFF,
                            OP.mult, OP.add)
            g2_ps = gtps.tile([2, B], F32)
            TE.matmul(g2_ps[:], sel2_sb[:], gthr[:], start=True, stop=True)
            g2 = gsb.tile([2, B], F32)
            V.tensor_copy(g2[:], g2_ps[:])
            tv_ps = gtps.tile([128, B], F32)
            TE.matmul(tv_ps[:], sel128_sb[:], g2[:], start=True, stop=True)
            V.tensor_copy(thrv[:], tv_ps[:])

    # ================= M = k^T v (local, per batch) =================
    with tc.tile_pool(name="mps", bufs=1, space="PSUM") as mps:
        for b in range(B):
            m_ps = mps.tile([128, 128], F32, name=f"mps{b}")
            for i in range(8):
                TE.matmul(m_ps[:], knat[:, b * 8 + i, :], vnat[:, b * 8 + i, :],
                          start=(i == 0), stop=(i == 7))
            # keep only the per-head diagonal 64-blocks (fp16 exact: <=1024)
            V.tensor_copy(m_sb[0:64, b, 0:64], m_ps[0:64, 0:64])
            SC.activation(m_sb[64:128, b, 64:128], m_ps[64:128, 64:128],
                          AF.Copy)

    # ============ apply: S = M_b^T q, spike at S >= 1.75 ============
    with tc.tile_pool(name="aps", bufs=1, space="PSUM") as aps:
        for b in range(B):
            for nh in range(2):
                n0 = b * NSEQ + nh * 512
                p = aps.tile([128, 512], F32, name=f"s{b}{nh}")
                TE.matmul(p[:], m_sb[:, b, :], sp["q"][:, n0:n0 + 512],
                          start=True, stop=True)
                V.tensor_scalar(xsp_cm[:, n0:n0 + 512], p[:],
                                thrv[:, b:b + 1], None, OP.is_ge)

    # ================= AllToAll: reshard to token-parallel =================
    for j in range(NCORES):
        DENG[j % 3].dma_start(
            a2a_pay[j * 128 * LTOK:(j + 1) * 128 * LTOK].rearrange(
                "(p n) -> p n", p=128),
            xsp_cm[:, j * LTOK:(j + 1) * LTOK])
    GP.collective_compute("AllToAll", OP.bypass,
                          ins=[a2a_pay.opt()], outs=[a2a_out.opt()],
                          replica_groups=[list(range(NCORES))])
    for j in range(NCORES):
        DENG[j % 3].dma_start(
            xsp_tok[:, j, :],
            a2a_out[j * 128 * LTOK:(j + 1) * 128 * LTOK].rearrange(
                "(p n) -> p n", p=128))

    # ================= projection (token-parallel) =================
    with tc.tile_pool(name="pstat", bufs=1) as pstat:
        with tc.tile_pool(name="ppps", bufs=1, space="PSUM") as ppps:
            pp = [ppps.tile([128, LTOK], F32, name=f"pp{i}") for i in range(8)]
            for mt in range(8):
                for kt in range(KT):
                    TE.matmul(pp[mt][:], wpT_sb[:, kt, mt, :], xsp_tok[:, kt, :],
                              start=(kt == 0), stop=(kt == KT - 1))
            stats = pstat.tile([128, 8, 6], F32)
            mv8 = pstat.tile([128, 8, 2], F32)
            for mt in range(8):
                V.bn_stats(stats[:, mt, :], pp[mt][:])
                V.bn_aggr(mv8[:, mt, :], stats[:, mt, :])
            s1 = pstat.tile([128, 8], F32)
            s2 = pstat.tile([128, 8], F32)
            V.tensor_scalar(s1[:], mv8[:, :, 0], float(LTOK), None, OP.mult)
            V.tensor_tensor(s2[:], mv8[:, :, 0], mv8[:, :, 0], OP.mult)
            V.tensor_tensor(s2[:], s2[:], mv8[:, :, 1], OP.add)
            V.tensor_scalar(s2[:], s2[:], float(LTOK), None, OP.mult)
            nc.sync.dma_start(
                st_pay[0:1024].rearrange("(p m) -> p m", p=128), s1[:])
            nc.sync.dma_start(
                st_pay[1024:2048].rearrange("(p m) -> p m", p=128), s2[:])
            GP.collective_compute("AllReduce", OP.add,
                                  ins=[st_pay.opt()], outs=[st_out.opt()],
                                  replica_groups=[list(range(NCORES))])
            g1 = pstat.tile([128, 8], F32)
            g2 = pstat.tile([128, 8], F32)
            nc.sync.dma_start(
                g1[:], st_out[0:1024].rearrange("(p m) -> p m", p=128))
            nc.scalar.dma_start(
                g2[:], st_out[1024:2048].rearrange("(p m) -> p m", p=128))
            meang = pstat.tile([128, 8], F32)
            V.tensor_scalar(meang[:], g1[:], 1.0 / TOK, None, OP.mult)
            varg = pstat.tile([128, 8], F32)
            V.tensor_scalar(varg[:], g2[:], 1.0 / TOK, None, OP.mult)
            msq2 = pstat.tile([128, 8], F32)
            V.tensor_tensor(msq2[:], meang[:], meang[:], OP.mult)
            V.tensor_tensor(varg[:], varg[:], msq2[:], OP.subtract)
            stdp = pstat.tile([128, 8], F32)
            SC.activation(stdp[:], varg[:], AF.Sqrt, bias=eps_sb[:])
            thrp = pstat.tile([128, 8], F32)
            V.tensor_tensor(thrp[:], stdp[:], tp_sb[:], OP.mult)
            V.tensor_tensor(thrp[:], thrp[:], meang[:], OP.add)
            osp = pstat.tile([128, 8, LTOK], BF16)
            for mt in range(8):
                V.tensor_scalar(osp[:, mt, :], pp[mt][:],
                                thrp[:, mt:mt + 1], None, OP.is_ge)
                DENG[mt % 3].dma_start(outT.ap()[mt, :, :], osp[:, mt, :])


def _tile_rows(a):
    # (8*128, N) -> (128, 8*N) so the SBUF [p, (t n)] load is contiguous
    n = a.shape[1]
    return np.ascontiguousarray(
        a.reshape(KT, 128, n).transpose(1, 0, 2).reshape(128, KT * n))


def _prep_inputs(inputs):
    x = np.asarray(inputs["x"], np.float32)
    xT = _tile_rows(x.reshape(TOK, D).T.astype(BF))
    Wg = np.asarray(inputs["Wg"], np.float64)
    wgr = (Wg.reshape(H, HD, H).sum(axis=1).T / 1024.0).astype(np.float32)
    wgr = np.ascontiguousarray(wgr)                     # [h, h']
    bgr = np.asarray(inputs["bg"], np.float32).reshape(H, 1)
    sel128 = np.zeros((2, 128), np.float32)
    sel128[0, 0:64] = 1.0
    sel128[1, 64:128] = 1.0
    idn = np.eye(128, dtype=np.float16)
    Wp = np.asarray(inputs["Wp"], np.float32).astype(BF)
    wpT = np.ascontiguousarray(
        Wp.reshape(8, 128, 8, 128).transpose(3, 2, 0, 1)).reshape(128, -1)
    gpf = np.asarray(inputs["gp"], np.float32)
    bepf = np.asarray(inputs["betap"], np.float32)
    tpv = (2.0 - bepf) / gpf                            # [1024] per c_out
    tp = np.ascontiguousarray(tpv.reshape(8, 128).T).astype(np.float32)
    in_maps = []
    for c in range(NCORES):
        sl = slice(CH * c, CH * c + CH)
        sel2T = np.zeros((2, H), np.float32)
        sel2T[0, 2 * c] = 1.0
        sel2T[1, 2 * c + 1] = 1.0
        sel2 = np.ascontiguousarray(sel2T.T)
        m = {"wgr": wgr, "bgr": bgr, "sel2T": sel2T, "sel2": sel2,
             "sel128": sel128, "idn": idn, "wpT": wpT, "tp": tp}
        for kt in range(KT):
            m[f"xt{kt}"] = np.ascontiguousarray(xT[:, kt * TOK:(kt + 1) * TOK])
        for nm in ("q", "k", "v"):
            W = np.asarray(inputs[f"W{nm}"], np.float32)
            m["w" + nm] = _tile_rows(W[sl, :].T.astype(BF))
            g = np.asarray(inputs[f"g{nm}"], np.float32)[sl]
            be = np.asarray(inputs[f"beta{nm}"], np.float32)[sl]
            m["t" + nm] = ((2.0 - be) / g).reshape(CH, 1).astype(np.float32)
        in_maps.append(m)
    return in_maps


def _assemble(results):
    out = np.empty((TOK, D), np.float32)
    for c in range(NCORES):
        o = np.asarray(results[c]["outT"], dtype=np.float32)  # [8, 128, 512]
        out[LTOK * c:LTOK * (c + 1), :] = \
            o.transpose(2, 0, 1).reshape(LTOK, D)
    return out.reshape(B, NSEQ, D)


def _run(inputs, trace=False):
    if "nc" not in _CACHE:
        _CACHE["nc"] = _build()
    nc = _CACHE["nc"]
    in_maps = _prep_inputs(inputs)
    res = run_bass_kernel_spmd(nc, in_maps, core_ids=list(range(NCORES)),
                               trace=trace)
    return _assemble(res.results), res


def kernel(**inputs) -> np.ndarray:
    out, _ = _run(inputs, trace=False)
    return out
